# revision 4
# baseline (speedup 1.0000x reference)
"""Trainium2 Bass kernel for nn_BiologicalWorkingMemory.

Pure data-parallel sharding: batch dim B=65536 split across 8 NeuronCores
(8192 batches each).  Per core, batches are laid out 128 per partition x T
batch-columns in the free dimension, processed in NBLK blocks.

Math notes (error budget ~2e-4 absolute, measured against the exact
sequential reference in fp64/numpy):
 - The reference's sequential in-place interference loop is replaced by a
   one-shot ("Jacobi") form: all pair similarities are computed from the
   decayed-memory Gram matrix.  Measured absmax deviation 1.8e-4.
 - Similarities are scale-invariant up to the 1e-6 epsilon, so the Gram is
   computed on UNDECAYED memory and the epsilon adjusted by 1/0.95^2.
 - Gram products run in bf16 (fp32 accumulate); all correction terms are
   O(0.01) relative, so bf16 error lands ~1e-4 absolute on m.
 - Decay, interference, gated write, refresh and capacity-halving fold into
   one per-slot linear combination  m_out[i] = sum_l E[i,l]*m[l] + beta_i*inp,
   evaluated as an 8x(10-padded) product + reduce per output slot.
"""

import sys

sys.path.insert(0, "/opt/trn_rl_repo")

import dataclasses

import numpy as np

import concourse.bass as bass
import concourse.tile as tile
from concourse import mybir
from concourse.bass_utils import run_bass_kernel_spmd

F32 = mybir.dt.float32
BF16 = mybir.dt.bfloat16
X = mybir.AxisListType.X
OP = mybir.AluOpType

PB = 128          # partitions (batches per partition-column)
T = 16            # batch columns per block
S = 8             # slots
D = 64            # slot dim
N_CORES = 8
BIG = 1.0e6
DECAY = 0.95
ACT_DECAY = 0.9
EPS_ADJ = 1e-6 / (DECAY * DECAY)   # epsilon for undecayed-Gram similarities


def _re(apv, dims):
    """Raw-AP helper: same tensor/offset/partition dim, custom free dims."""
    return dataclasses.replace(apv, ap=[apv.ap[0]] + [list(d) for d in dims])



MAX_WAITS = 1


class _TC(tile.TileContext):
    """Works around this walrus build's sync-wait-per-instruction limit: the
    kernel-tail drain gets all global-clock sem waits on one SP Drain; split
    them one per Drain instruction."""

    def _drain_and_barrier(self, tick_clock, wait_clock):
        nc = self.nc
        drain_inst = nc.sync.drain()
        wait_clock.add_sem_waits(
            drain_inst.ins, tile.ScopedClock({None: tick_clock.global_clock})
        )
        si = drain_inst.ins.sync_info
        w = list(si.on_wait or []) if si else []
        if len(w) > MAX_WAITS:
            si.on_wait = w[:MAX_WAITS]
            rest = w[MAX_WAITS:]
            while rest:
                d2 = nc.sync.drain()
                si2 = d2.ins.sync_info
                if si2 is None:
                    d2.ins.sync_info = mybir.SyncInfo(on_wait=rest[:MAX_WAITS], on_update=[])
                else:
                    si2.on_wait = rest[:MAX_WAITS]
                rest = rest[MAX_WAITS:]
        nc.all_engine_barrier()
        assert self.sems is not None
        popped = nc._tile_sem_poison_stack.pop()
        assert popped is self._sem_poison
        nc.clear_and_free_semaphores(list(self.sems.allocated().values()))
        nc.all_engine_barrier()


def build(nblk, dtv):
    Bc = PB * T * nblk
    nc = bass.Bass("TRN2", target_bir_lowering=False)

    di = {}
    di["memory_slots"] = nc.dram_tensor("memory_slots", [Bc, S, D], F32, kind="ExternalInput")
    for nm in ("slot_activities", "slot_gates", "gate_thresholds", "refresh_strengths",
               "maintenance_currents", "gate_signals", "refresh_signals"):
        di[nm] = nc.dram_tensor(nm, [Bc, S], F32, kind="ExternalInput")
    di["interference_matrix"] = nc.dram_tensor("interference_matrix", [Bc, S, S], F32, kind="ExternalInput")
    di["inputs"] = nc.dram_tensor("inputs", [Bc, D], F32, kind="ExternalInput")

    do = {}
    do["m"] = nc.dram_tensor("out_m", [Bc, S, D], F32, kind="ExternalOutput")
    do["a"] = nc.dram_tensor("out_a", [Bc, S], F32, kind="ExternalOutput")
    do["g"] = nc.dram_tensor("out_g", [Bc, S], F32, kind="ExternalOutput")
    do["mc"] = nc.dram_tensor("out_mc", [Bc, S], F32, kind="ExternalOutput")
    do["ml"] = nc.dram_tensor("out_ml", [Bc], F32, kind="ExternalOutput")
    do["ta"] = nc.dram_tensor("out_ta", [Bc], F32, kind="ExternalOutput")
    do["ms"] = nc.dram_tensor("out_ms", [Bc], F32, kind="ExternalOutput")

    with _TC(nc) as tc:
        with (
            tc.tile_pool(name="big", bufs=1) as big,
            tc.tile_pool(name="mid", bufs=1) as mid,
            tc.tile_pool(name="ins", bufs=2) as insp,
            tc.tile_pool(name="outs", bufs=1) as outp,
        ):
            V = nc.vector
            A = nc.scalar
            for b in range(nblk):
                base = b * PB * T

                def dview(t_, extra="s"):
                    sl = t_[base:base + PB * T]
                    if extra == "sd":
                        return sl.rearrange("(p t) s d -> p t s d", t=T)
                    if extra == "s":
                        return sl.rearrange("(p t) s -> p t s", t=T)
                    if extra == "d":
                        return sl.rearrange("(p t) d -> p t d", t=T)
                    return sl.rearrange("(p t) -> p t", t=T)

                # ---- tiles ----
                m10 = big.tile([PB, T, 10, D], F32, tag="m10", name="m10")
                P = big.tile([PB, T, D, 10], F32, tag="P", name="P")
                mbf = big.tile([PB, T, S, D], BF16, tag="mbf", name="mbf")
                om = big.tile([PB, T, S, D], F32, tag="om", name="om")
                ws = [mid.tile([PB, T, S, S], F32, tag=f"ws{k}", name=f"ws{k}") for k in range(8)]
                G, Gt, Gs, wa, wb, wc, wd, we = ws
                sp = mid.tile([PB, 24, T, S], F32, tag="sp", name="sp")     # [T,8] workspaces
                pads = mid.tile([PB, 3, T, 16], F32, tag="pads", name="pads")
                sc = mid.tile([PB, 24, T], F32, tag="sc", name="sc")        # [T] workspaces
                ism = insp.tile([PB, 7, T, S], F32, tag="ism", name="ism")
                iW = insp.tile([PB, T, S, S], F32, tag="iW", name="iW")

                t_act, t_sg, t_thr, t_rstr, t_mcur, t_gsig, t_rsig = (ism[:, k] for k in range(7))

                # ---- loads ----
                nc.sync.dma_start(out=m10[:, :, 0:S, :], in_=dview(di["memory_slots"], "sd"))
                nc.sync.dma_start(out=m10[:, :, 8, :], in_=dview(di["inputs"], "d"))
                V.memset(m10[:, :, 9, :], 0.0)
                for k, nm in enumerate(("slot_activities", "slot_gates", "gate_thresholds",
                                        "refresh_strengths", "maintenance_currents",
                                        "gate_signals", "refresh_signals")):
                    nc.sync.dma_start(out=ism[:, k], in_=dview(di[nm], "s"))
                nc.sync.dma_start(out=iW, in_=dview(di["interference_matrix"], "sd"))

                # ---- slot selection (pre-decay activities) ----
                avail, t1, masked, srcarr, ohw, onehot = (sp[:, k] for k in range(6))
                minm, anyav, minall, tv, gs_, thr_, wr_, wgs3 = (sc[:, k] for k in range(8))
                V.tensor_scalar(avail, t_act, 0.2, None, op0=OP.is_lt)
                V.tensor_scalar(t1, avail, -BIG, BIG, op0=OP.mult, op1=OP.add)
                V.tensor_add(masked, t_act, t1)
                V.tensor_reduce(minm, masked, axis=X, op=OP.min)
                V.tensor_reduce(anyav, avail, axis=X, op=OP.max)
                V.tensor_reduce(minall, t_act, axis=X, op=OP.min)
                d1s, d2s = sc[:, 8], sc[:, 9]
                V.tensor_sub(d1s, minm, minall)
                V.tensor_mul(d2s, d1s, anyav)
                V.tensor_add(tv, minall, d2s)
                anyav_b = _re(anyav, [anyav.ap[1:][0], [0, S]])
                V.tensor_mul(srcarr, t1, anyav_b)          # srcarr tmp = t1*anyav
                V.tensor_add(srcarr, t_act, srcarr)
                # first-index one-hot of (srcarr == tv)
                V.memset(pads, 0.0)
                raw = pads[:, 0, :, 4:12]
                tv_b = _re(tv, [tv.ap[1:][0], [0, S]])
                V.tensor_tensor(raw, srcarr, tv_b, op=OP.is_equal)
                p1 = pads[:, 1, :, 4:12]
                rawm1 = pads[:, 0, :, 3:11]
                V.tensor_add(p1, raw, rawm1)
                p2 = pads[:, 2, :, 4:12]
                V.tensor_add(p2, p1, pads[:, 1, :, 2:10])
                p3 = pads[:, 1, :, 4:12]   # overwrite pad1
                V.tensor_add(p3, p2, pads[:, 2, :, 0:8])
                ieq = sp[:, 6]
                V.tensor_scalar(ieq, p3, 1.0, None, op0=OP.is_equal)
                V.tensor_mul(onehot, raw, ieq)

                # ---- gate update + gather ----
                og = sp[:, 7]
                clipg, sg7 = sp[:, 8], sp[:, 9]
                V.tensor_scalar(clipg, t_gsig, 0.0, 1.0, op0=OP.max, op1=OP.min)
                V.tensor_scalar_mul(sg7, t_sg, 0.7)
                V.scalar_tensor_tensor(og, clipg, 0.3, sg7, op0=OP.mult, op1=OP.add)
                mws = sp[:, 10]
                V.tensor_mul(mws, onehot, og)
                V.tensor_reduce(gs_, mws, axis=X, op=OP.add)
                V.tensor_mul(mws, onehot, t_thr)
                V.tensor_reduce(thr_, mws, axis=X, op=OP.add)
                V.tensor_tensor(wr_, gs_, thr_, op=OP.is_gt)
                wr_b = _re(wr_, [wr_.ap[1:][0], [0, S]])
                V.tensor_mul(ohw, onehot, wr_b)
                V.tensor_mul(wgs3, gs_, wr_)
                V.tensor_scalar_mul(wgs3, wgs3, 0.3)

                # ---- activity pipeline ----
                a_dec, actm, ru, rmask, rs, fplus, cw, onemcw = (sp[:, k] for k in range(11, 19))
                V.tensor_scalar_mul(a_dec, t_act, ACT_DECAY)
                V.tensor_scalar(actm, a_dec, 0.1, None, op0=OP.is_gt)
                V.tensor_scalar(ru, t_rsig, 0.0, 1.0, op0=OP.max, op1=OP.min)
                V.tensor_scalar(rmask, ru, 0.1, None, op0=OP.is_gt)
                V.tensor_mul(rs, t_rstr, ru)
                V.tensor_mul(rs, rs, rmask)
                V.tensor_scalar(fplus, rs, 1.0, None, op0=OP.add)
                wgs3_b = _re(wgs3, [wgs3.ap[1:][0], [0, S]])
                V.tensor_mul(cw, ohw, wgs3_b)
                V.tensor_scalar(onemcw, cw, -1.0, 1.0, op0=OP.mult, op1=OP.add)

                # ninp = ||inputs||
                sqv = P[:, :, :, 0]
                ninp = sc[:, 10]
                V.tensor_mul(sqv, m10[:, :, 8, :], m10[:, :, 8, :])
                V.tensor_reduce(ninp, sqv, axis=X, op=OP.add)
                A.sqrt(ninp, ninp)

                u1, u2, a1, a2 = sp[:, 19], sp[:, 10], sp[:, 8], sp[:, 9]
                ninp_b = _re(ninp, [ninp.ap[1:][0], [0, S]])
                V.tensor_sub(u1, ninp_b, a_dec)
                V.tensor_mul(u2, u1, ohw)
                V.tensor_add(a1, a_dec, u2)
                V.tensor_add(a2, a1, rs)

                active, v1, m2t = sp[:, 0], sp[:, 20], sp[:, 21]
                nact, ndeact = sc[:, 11], sc[:, 12]
                V.tensor_scalar(active, a2, 0.1, None, op0=OP.is_gt)
                V.tensor_reduce(nact, active, axis=X, op=OP.add)
                V.tensor_scalar(ndeact, nact, 4.0, 0.0, op0=OP.subtract, op1=OP.max)
                V.tensor_scalar(v1, active, -BIG, BIG, op0=OP.mult, op1=OP.add)
                V.tensor_add(m2t, a2, v1)
                # pairwise rank
                m2_i = _re(m2t, [m2t.ap[1:][0], [1, S], [0, S]])
                m2_j = _re(m2t, [m2t.ap[1:][0], [0, S], [1, S]])
                V.tensor_tensor(wa, m2_j, m2_i, op=OP.is_lt)   # [t,s,s'] = m2[s'] < m2[s]
                rank, r1, deact, dfa, df = sp[:, 22], sp[:, 23], sp[:, 10], sp[:, 20], sp[:, 21]
                V.tensor_reduce(rank, wa, axis=X, op=OP.add)
                nd_b = _re(ndeact, [ndeact.ap[1:][0], [0, S]])
                V.tensor_tensor(r1, rank, nd_b, op=OP.is_lt)
                V.tensor_mul(deact, r1, active)
                V.tensor_scalar(dfa, deact, -0.5, 1.0, op0=OP.mult, op1=OP.add)
                V.tensor_scalar(df, deact, -0.3, 1.0, op0=OP.mult, op1=OP.add)

                # alpha/beta
                alpha, beta = sp[:, 22], sp[:, 23]  # rank,r1 dead now
                V.tensor_mul(alpha, onemcw, fplus)
                V.tensor_mul(alpha, alpha, df)
                V.tensor_scalar_mul(alpha, alpha, DECAY)
                V.tensor_mul(beta, cw, fplus)
                V.tensor_mul(beta, beta, df)

                # a out / mc out / scalar outs
                oa = insp.tile([PB, T, S], F32, tag="oa", name="oa")
                omc = insp.tile([PB, T, S], F32, tag="omc", name="omc")
                V.tensor_tensor(oa, a2, dfa, op=OP.mult)
                k1, mca, mci, dd1 = sp[:, 19], sp[:, 20], sp[:, 10], sp[:, 19]
                V.tensor_scalar_mul(k1, a2, 0.05 * dtv)
                V.scalar_tensor_tensor(mca, t_mcur, 1.0 - 0.1 * dtv, k1, op0=OP.mult, op1=OP.add)
                V.tensor_scalar_mul(mci, t_mcur, 0.95)
                V.tensor_sub(dd1, mca, mci)
                V.tensor_mul(dd1, dd1, active)
                V.tensor_add(omc, mci, dd1)

                afm, oml, ota, oms = sp[:, 20], sc[:, 13], sc[:, 14], sc[:, 15]
                V.tensor_scalar(afm, oa, 0.1, None, op0=OP.is_gt)
                V.tensor_reduce(oml, afm, axis=X, op=OP.add)
                V.tensor_reduce(ota, oa, axis=X, op=OP.add)
                V.tensor_reduce(oms, omc, axis=X, op=OP.add)
                V.tensor_scalar_mul(oms, oms, 1.0 / S)

                # ---- Gram (bf16) ----
                A.copy(mbf, m10[:, :, 0:S, :])
                V.memset(G, 0.0)
                pr = mid.tile([PB, T, D], BF16, tag="pr", name="pr")
                for i in range(S):
                    for j in range(i, S):
                        V.tensor_mul(pr, mbf[:, :, i, :], mbf[:, :, j, :])
                        V.tensor_reduce(G[:, :, i, j], pr, axis=X, op=OP.add)
                V.tensor_copy(Gt, G.rearrange("p t i j -> p t j i"))
                V.tensor_add(Gs, G, Gt)   # full symmetric, diag = 2*diag(G) but unused
                dg0 = G[:, :, 0, 0]
                dg_i = _re(dg0, [dg0.ap[1:][0], [S + 1, S], [0, S]])
                dg_j = _re(dg0, [dg0.ap[1:][0], [0, S], [S + 1, S]])
                V.tensor_tensor(wa, dg_i, dg_j, op=OP.mult)
                A.sqrt(wb, wa)
                V.tensor_scalar(wb, wb, EPS_ADJ, None, op0=OP.add)
                V.reciprocal(wc, wb)
                # zero the diagonal of Gs so C diag = 0
                gsd = Gs[:, :, 0, 0]
                V.memset(_re(gsd, [gsd.ap[1:][0], [S + 1, S]]), 0.0)
                V.tensor_mul(wc, Gs, wc)                  # sim (diag 0)
                V.tensor_scalar_mul(wd, iW, -0.1)
                actm_i = _re(actm, [actm.ap[1:][0], [1, S], [0, S]])
                actm_j = _re(actm, [actm.ap[1:][0], [0, S], [1, S]])
                V.tensor_mul(wd, wd, actm_i)
                V.tensor_mul(wd, wd, actm_j)
                V.tensor_mul(we, wd, wc)                  # E offdiag = -0.1*W*cond*sim
                wed = we[:, :, 0, 0]
                V.tensor_scalar(_re(wed, [wed.ap[1:][0], [S + 1, S]]),
                                _re(wed, [wed.ap[1:][0], [S + 1, S]]),
                                1.0, None, op0=OP.add)    # E diag = 1
                E9 = mid.tile([PB, T, S, 10], F32, tag="E9", name="E9")
                alpha_b = _re(alpha, [alpha.ap[1:][0], [1, S], [0, S]])
                V.tensor_mul(E9[:, :, :, 0:S], we, alpha_b)
                V.tensor_copy(E9[:, :, :, 8], beta)
                V.memset(E9[:, :, :, 9], 0.0)

                # ---- reconstruction ----
                m_dl = m10.rearrange("p t l d -> p t d l")
                for i in range(S):
                    e_row = E9[:, :, i, :]
                    e_b = _re(e_row, [e_row.ap[1:][0], [0, D], [1, 10]])
                    V.tensor_mul(P, m_dl, e_b)
                    V.tensor_reduce(om[:, :, i, :], P, axis=X, op=OP.add)

                # ---- stores ----
                nc.sync.dma_start(out=dview(do["m"], "sd"), in_=om)
                nc.sync.dma_start(out=dview(do["a"], "s"), in_=oa)
                nc.sync.dma_start(out=dview(do["g"], "s"), in_=og)
                nc.sync.dma_start(out=dview(do["mc"], "s"), in_=omc)
                nc.sync.dma_start(out=dview(do["ml"], "x"), in_=oml)
                nc.sync.dma_start(out=dview(do["ta"], "x"), in_=ota)
                nc.sync.dma_start(out=dview(do["ms"], "x"), in_=oms)
    return nc



def _split_waits(nc, max_waits=1):
    """This walrus build accepts at most one sync-wait per instruction; hoist
    excess waits onto dedicated Drain carriers inserted just before."""
    wid = [0]
    for blk in nc.m.functions[0].blocks:
        out = []
        for inst in blk.instructions:
            si = inst.sync_info
            w = list(si.on_wait or []) if si else []
            if len(w) > max_waits:
                rest, keep = w[:-max_waits], w[-max_waits:]
                for chunk_start in range(0, len(rest), max_waits):
                    chunk = rest[chunk_start:chunk_start + max_waits]
                    d = mybir.InstDrain(name=f"WSPLIT-{wid[0]}", ins=[], outs=[])
                    wid[0] += 1
                    d.engine = inst.engine
                    d.sync_info = mybir.SyncInfo(on_wait=chunk, on_update=[])
                    out.append(d)
                si.on_wait = keep
            out.append(inst)
        blk.instructions[:] = out


_NC_CACHE = {}


def _run(inputs, nblk, trace=False):
    dtv = float(np.asarray(inputs["dt"]))
    key = (nblk, dtv)
    if key not in _NC_CACHE:
        nc_new = build(nblk, dtv)
        _split_waits(nc_new)
        _NC_CACHE[key] = nc_new
    nc = _NC_CACHE[key]
    Bc = PB * T * nblk
    B = Bc * N_CORES
    names = ["memory_slots", "slot_activities", "slot_gates", "gate_thresholds",
             "refresh_strengths", "maintenance_currents", "gate_signals",
             "refresh_signals", "interference_matrix", "inputs"]
    in_maps = []
    for c in range(N_CORES):
        sl = slice(c * Bc, (c + 1) * Bc)
        in_maps.append({nm: np.ascontiguousarray(np.asarray(inputs[nm][sl], dtype=np.float32))
                        for nm in names})
    res = run_bass_kernel_spmd(nc, in_maps, core_ids=list(range(N_CORES)), trace=trace)
    outs = []
    for nm, shp in (("out_m", (B, S, D)), ("out_a", (B, S)), ("out_g", (B, S)),
                    ("out_mc", (B, S)), ("out_ml", (B,)), ("out_ta", (B,)), ("out_ms", (B,))):
        outs.append(np.concatenate([res.results[c][nm] for c in range(N_CORES)], axis=0).reshape(shp))
    return tuple(outs), res


def kernel(**inputs):
    (m, a, g, mc, ml, ta, ms), _ = _run(inputs, nblk=4)
    return m, a, g, mc, ml, ta, ms


# revision 8
# speedup vs baseline: 6.5514x; 6.5514x over previous
"""Trainium2 Bass kernel for nn_BiologicalWorkingMemory.

Pure data-parallel sharding: batch dim B=65536 split across 8 NeuronCores
(8192 batches each).  Per core, batches are laid out 128 per partition x T
batch-columns in the free dimension, processed in NBLK blocks.

Math notes (error budget ~2e-4 absolute, measured against the exact
sequential reference in fp64/numpy):
 - The reference's sequential in-place interference loop is replaced by a
   one-shot ("Jacobi") form: all pair similarities are computed from the
   decayed-memory Gram matrix.  Measured absmax deviation 1.8e-4.
 - Similarities are scale-invariant up to the 1e-6 epsilon, so the Gram is
   computed on UNDECAYED memory and the epsilon adjusted by 1/0.95^2.
 - Gram products run in bf16 (fp32 accumulate); all correction terms are
   O(0.01) relative, so bf16 error lands ~1e-4 absolute on m.
 - Decay, interference, gated write, refresh and capacity-halving fold into
   one per-slot linear combination  m_out[i] = sum_l E[i,l]*m[l] + beta_i*inp,
   evaluated as an 8x(10-padded) product + reduce per output slot.
"""

import sys

sys.path.insert(0, "/opt/trn_rl_repo")

import dataclasses

import numpy as np

import concourse.bass as bass
import concourse.tile as tile
from concourse import mybir
from concourse.bass_utils import run_bass_kernel_spmd

F32 = mybir.dt.float32
BF16 = mybir.dt.bfloat16
X = mybir.AxisListType.X
OP = mybir.AluOpType

PB = 128          # partitions (batches per partition-column)
T = 16            # batch columns per block
S = 8             # slots
D = 64            # slot dim
N_CORES = 8
BIG = 1.0e6
DECAY = 0.95
ACT_DECAY = 0.9
EPS_ADJ = 1e-6 / (DECAY * DECAY)   # epsilon for undecayed-Gram similarities


def _re(apv, dims):
    """Raw-AP helper: same tensor/offset/partition dim, custom free dims."""
    return dataclasses.replace(apv, ap=[apv.ap[0]] + [list(d) for d in dims])



MAX_WAITS = 1


class _TC(tile.TileContext):
    """Works around this walrus build's sync-wait-per-instruction limit: the
    kernel-tail drain gets all global-clock sem waits on one SP Drain; split
    them one per Drain instruction."""

    def _drain_and_barrier(self, tick_clock, wait_clock):
        nc = self.nc
        drain_inst = nc.sync.drain()
        wait_clock.add_sem_waits(
            drain_inst.ins, tile.ScopedClock({None: tick_clock.global_clock})
        )
        si = drain_inst.ins.sync_info
        w = list(si.on_wait or []) if si else []
        if len(w) > MAX_WAITS:
            si.on_wait = w[:MAX_WAITS]
            rest = w[MAX_WAITS:]
            while rest:
                d2 = nc.sync.drain()
                si2 = d2.ins.sync_info
                if si2 is None:
                    d2.ins.sync_info = mybir.SyncInfo(on_wait=rest[:MAX_WAITS], on_update=[])
                else:
                    si2.on_wait = rest[:MAX_WAITS]
                rest = rest[MAX_WAITS:]
        nc.all_engine_barrier()
        assert self.sems is not None
        popped = nc._tile_sem_poison_stack.pop()
        assert popped is self._sem_poison
        nc.clear_and_free_semaphores(list(self.sems.allocated().values()))
        nc.all_engine_barrier()


def build(nblk, dtv):
    Bc = PB * T * nblk
    nc = bass.Bass("TRN2", target_bir_lowering=False)

    di = {}
    di["memory_slots"] = nc.dram_tensor("memory_slots", [Bc, S, D], F32, kind="ExternalInput")
    for nm in ("slot_activities", "slot_gates", "gate_thresholds", "refresh_strengths",
               "maintenance_currents", "gate_signals", "refresh_signals"):
        di[nm] = nc.dram_tensor(nm, [Bc, S], F32, kind="ExternalInput")
    di["interference_matrix"] = nc.dram_tensor("interference_matrix", [Bc, S, S], F32, kind="ExternalInput")
    di["inputs"] = nc.dram_tensor("inputs", [Bc, D], F32, kind="ExternalInput")

    do = {}
    do["m"] = nc.dram_tensor("out_m", [Bc, S, D], F32, kind="ExternalOutput")
    do["a"] = nc.dram_tensor("out_a", [Bc, S], F32, kind="ExternalOutput")
    do["g"] = nc.dram_tensor("out_g", [Bc, S], F32, kind="ExternalOutput")
    do["mc"] = nc.dram_tensor("out_mc", [Bc, S], F32, kind="ExternalOutput")
    do["ml"] = nc.dram_tensor("out_ml", [Bc], F32, kind="ExternalOutput")
    do["ta"] = nc.dram_tensor("out_ta", [Bc], F32, kind="ExternalOutput")
    do["ms"] = nc.dram_tensor("out_ms", [Bc], F32, kind="ExternalOutput")

    with _TC(nc) as tc:
        with (
            tc.tile_pool(name="big", bufs=1) as big,
            tc.tile_pool(name="mid", bufs=1) as mid,
            tc.tile_pool(name="ins", bufs=2) as insp,
            tc.tile_pool(name="outs", bufs=1) as outp,
        ):
            V = nc.vector
            A = nc.scalar
            for b in range(nblk):
                base = b * PB * T

                def dview(t_, extra="s"):
                    sl = t_[base:base + PB * T]
                    if extra == "sd":
                        return sl.rearrange("(p t) s d -> p t s d", t=T)
                    if extra == "s":
                        return sl.rearrange("(p t) s -> p t s", t=T)
                    if extra == "d":
                        return sl.rearrange("(p t) d -> p t d", t=T)
                    return sl.rearrange("(p t) -> p t", t=T)

                # ---- tiles ----
                m10 = big.tile([PB, T, 10, D], F32, tag="m10", name="m10")
                mbf = big.tile([PB, T, S, D], BF16, tag="P", bufs=2, name="mbf")
                ws = [mid.tile([PB, T, S, S], F32, tag=f"ws{k}", name=f"ws{k}") for k in range(8)]
                G, Gt, Gs, wa, wb, wc, wd, we = ws
                sp = mid.tile([PB, 24, T, S], F32, tag="sp", name="sp")     # [T,8] workspaces
                pads = mid.tile([PB, 3, T, 16], F32, tag="pads", name="pads")
                sc = mid.tile([PB, 24, T], F32, tag="sc", name="sc")        # [T] workspaces
                ism = insp.tile([PB, 7, T, S], F32, tag="ism", name="ism")
                iW = insp.tile([PB, T, S, S], F32, tag="iW", name="iW")

                t_act, t_sg, t_thr, t_rstr, t_mcur, t_gsig, t_rsig = (ism[:, k] for k in range(7))

                # ---- loads ----
                nc.sync.dma_start(out=m10[:, :, 0:S, :], in_=dview(di["memory_slots"], "sd"))
                nc.sync.dma_start(out=m10[:, :, 8, :], in_=dview(di["inputs"], "d"))
                V.memset(m10[:, :, 9, :], 0.0)
                for k, nm in enumerate(("slot_activities", "slot_gates", "gate_thresholds",
                                        "refresh_strengths", "maintenance_currents",
                                        "gate_signals", "refresh_signals")):
                    nc.sync.dma_start(out=ism[:, k], in_=dview(di[nm], "s"))
                nc.sync.dma_start(out=iW, in_=dview(di["interference_matrix"], "sd"))

                # ---- slot selection (pre-decay activities) ----
                avail, t1, masked, srcarr, ohw, onehot = (sp[:, k] for k in range(6))
                minm, anyav, minall, tv, gs_, thr_, wr_, wgs3 = (sc[:, k] for k in range(8))
                V.tensor_scalar(avail, t_act, 0.2, None, op0=OP.is_lt)
                V.tensor_scalar(t1, avail, -BIG, BIG, op0=OP.mult, op1=OP.add)
                V.tensor_add(masked, t_act, t1)
                V.tensor_reduce(minm, masked, axis=X, op=OP.min)
                V.tensor_reduce(anyav, avail, axis=X, op=OP.max)
                V.tensor_reduce(minall, t_act, axis=X, op=OP.min)
                d1s, d2s = sc[:, 8], sc[:, 9]
                V.tensor_sub(d1s, minm, minall)
                V.tensor_mul(d2s, d1s, anyav)
                V.tensor_add(tv, minall, d2s)
                anyav_b = _re(anyav, [anyav.ap[1:][0], [0, S]])
                V.tensor_mul(srcarr, t1, anyav_b)          # srcarr tmp = t1*anyav
                V.tensor_add(srcarr, t_act, srcarr)
                # first-index one-hot of (srcarr == tv)
                V.memset(pads, 0.0)
                raw = pads[:, 0, :, 4:12]
                tv_b = _re(tv, [tv.ap[1:][0], [0, S]])
                V.tensor_tensor(raw, srcarr, tv_b, op=OP.is_equal)
                p1 = pads[:, 1, :, 4:12]
                rawm1 = pads[:, 0, :, 3:11]
                V.tensor_add(p1, raw, rawm1)
                p2 = pads[:, 2, :, 4:12]
                V.tensor_add(p2, p1, pads[:, 1, :, 2:10])
                p3 = pads[:, 1, :, 4:12]   # overwrite pad1
                V.tensor_add(p3, p2, pads[:, 2, :, 0:8])
                ieq = sp[:, 6]
                V.tensor_scalar(ieq, p3, 1.0, None, op0=OP.is_equal)
                V.tensor_mul(onehot, raw, ieq)

                # ---- gate update + gather ----
                og = sp[:, 7]
                clipg, sg7 = sp[:, 8], sp[:, 9]
                V.tensor_scalar(clipg, t_gsig, 0.0, 1.0, op0=OP.max, op1=OP.min)
                V.tensor_scalar_mul(sg7, t_sg, 0.7)
                V.scalar_tensor_tensor(og, clipg, 0.3, sg7, op0=OP.mult, op1=OP.add)
                mws = sp[:, 10]
                V.tensor_mul(mws, onehot, og)
                V.tensor_reduce(gs_, mws, axis=X, op=OP.add)
                V.tensor_mul(mws, onehot, t_thr)
                V.tensor_reduce(thr_, mws, axis=X, op=OP.add)
                V.tensor_tensor(wr_, gs_, thr_, op=OP.is_gt)
                wr_b = _re(wr_, [wr_.ap[1:][0], [0, S]])
                V.tensor_mul(ohw, onehot, wr_b)
                V.tensor_mul(wgs3, gs_, wr_)
                V.tensor_scalar_mul(wgs3, wgs3, 0.3)

                # ---- activity pipeline ----
                a_dec, actm, ru, rmask, rs, fplus, cw, onemcw = (sp[:, k] for k in range(11, 19))
                V.tensor_scalar_mul(a_dec, t_act, ACT_DECAY)
                V.tensor_scalar(actm, a_dec, 0.1, None, op0=OP.is_gt)
                V.tensor_scalar(ru, t_rsig, 0.0, 1.0, op0=OP.max, op1=OP.min)
                V.tensor_scalar(rmask, ru, 0.1, None, op0=OP.is_gt)
                V.tensor_mul(rs, t_rstr, ru)
                V.tensor_mul(rs, rs, rmask)
                V.tensor_scalar(fplus, rs, 1.0, None, op0=OP.add)
                wgs3_b = _re(wgs3, [wgs3.ap[1:][0], [0, S]])
                V.tensor_mul(cw, ohw, wgs3_b)
                V.tensor_scalar(onemcw, cw, -1.0, 1.0, op0=OP.mult, op1=OP.add)

                # ninp = ||inputs||
                sqv = _re(wa[:, :, 0, 0], [[S * S, T], [1, S * S]])
                ninp = sc[:, 10]
                V.tensor_mul(sqv, m10[:, :, 8, :], m10[:, :, 8, :])
                V.tensor_reduce(ninp, sqv, axis=X, op=OP.add)
                A.sqrt(ninp, ninp)

                u1, u2, a1, a2 = sp[:, 19], sp[:, 10], sp[:, 8], sp[:, 9]
                ninp_b = _re(ninp, [ninp.ap[1:][0], [0, S]])
                V.tensor_sub(u1, ninp_b, a_dec)
                V.tensor_mul(u2, u1, ohw)
                V.tensor_add(a1, a_dec, u2)
                V.tensor_add(a2, a1, rs)

                active, v1, m2t = sp[:, 0], sp[:, 20], sp[:, 21]
                nact, ndeact = sc[:, 11], sc[:, 12]
                V.tensor_scalar(active, a2, 0.1, None, op0=OP.is_gt)
                V.tensor_reduce(nact, active, axis=X, op=OP.add)
                V.tensor_scalar(ndeact, nact, 4.0, 0.0, op0=OP.subtract, op1=OP.max)
                V.tensor_scalar(v1, active, -BIG, BIG, op0=OP.mult, op1=OP.add)
                V.tensor_add(m2t, a2, v1)
                # pairwise rank
                m2_i = _re(m2t, [m2t.ap[1:][0], [1, S], [0, S]])
                m2_j = _re(m2t, [m2t.ap[1:][0], [0, S], [1, S]])
                V.tensor_tensor(wa, m2_j, m2_i, op=OP.is_lt)   # [t,s,s'] = m2[s'] < m2[s]
                rank, r1, deact, dfa, df = sp[:, 22], sp[:, 23], sp[:, 10], sp[:, 20], sp[:, 21]
                V.tensor_reduce(rank, wa, axis=X, op=OP.add)
                nd_b = _re(ndeact, [ndeact.ap[1:][0], [0, S]])
                V.tensor_tensor(r1, rank, nd_b, op=OP.is_lt)
                V.tensor_mul(deact, r1, active)
                V.tensor_scalar(dfa, deact, -0.5, 1.0, op0=OP.mult, op1=OP.add)
                V.tensor_scalar(df, deact, -0.3, 1.0, op0=OP.mult, op1=OP.add)

                # alpha/beta
                alpha, beta = sp[:, 22], sp[:, 23]  # rank,r1 dead now
                V.tensor_mul(alpha, onemcw, fplus)
                V.tensor_mul(alpha, alpha, df)
                V.tensor_scalar_mul(alpha, alpha, DECAY)
                V.tensor_mul(beta, cw, fplus)
                V.tensor_mul(beta, beta, df)

                # a out / mc out / scalar outs
                oa = insp.tile([PB, T, S], F32, tag="oa", name="oa")
                omc = insp.tile([PB, T, S], F32, tag="omc", name="omc")
                V.tensor_tensor(oa, a2, dfa, op=OP.mult)
                k1, mca, mci, dd1 = sp[:, 19], sp[:, 20], sp[:, 10], sp[:, 19]
                V.tensor_scalar_mul(k1, a2, 0.05 * dtv)
                V.scalar_tensor_tensor(mca, t_mcur, 1.0 - 0.1 * dtv, k1, op0=OP.mult, op1=OP.add)
                V.tensor_scalar_mul(mci, t_mcur, 0.95)
                V.tensor_sub(dd1, mca, mci)
                V.tensor_mul(dd1, dd1, active)
                V.tensor_add(omc, mci, dd1)

                afm, oml, ota, oms = sp[:, 20], sc[:, 13], sc[:, 14], sc[:, 15]
                V.tensor_scalar(afm, oa, 0.1, None, op0=OP.is_gt)
                V.tensor_reduce(oml, afm, axis=X, op=OP.add)
                V.tensor_reduce(ota, oa, axis=X, op=OP.add)
                V.tensor_reduce(oms, omc, axis=X, op=OP.add)
                V.tensor_scalar_mul(oms, oms, 1.0 / S)

                # ---- Gram (bf16) ----
                A.copy(mbf, m10[:, :, 0:S, :])
                V.memset(G, 0.0)
                pr = mid.tile([PB, T, D], BF16, tag="pr", name="pr")
                pidx = 0
                prg = mid.tile([PB, T, D], BF16, tag="prg", name="prg")
                for i in range(S):
                    for j in range(i, S):
                        if pidx % 3 == 2:
                            nc.gpsimd.tensor_mul(prg, mbf[:, :, i, :], mbf[:, :, j, :])
                            V.tensor_reduce(G[:, :, i, j], prg, axis=X, op=OP.add)
                        else:
                            V.tensor_mul(pr, mbf[:, :, i, :], mbf[:, :, j, :])
                            V.tensor_reduce(G[:, :, i, j], pr, axis=X, op=OP.add)
                        pidx += 1
                V.tensor_copy(Gt, G.rearrange("p t i j -> p t j i"))
                V.tensor_add(Gs, G, Gt)   # full symmetric, diag = 2*diag(G) but unused
                dg0 = G[:, :, 0, 0]
                dg_i = _re(dg0, [dg0.ap[1:][0], [S + 1, S], [0, S]])
                dg_j = _re(dg0, [dg0.ap[1:][0], [0, S], [S + 1, S]])
                V.tensor_tensor(wa, dg_i, dg_j, op=OP.mult)
                A.sqrt(wb, wa)
                V.tensor_scalar(wb, wb, EPS_ADJ, None, op0=OP.add)
                V.reciprocal(wc, wb)
                # zero the diagonal of Gs so C diag = 0
                gsd = Gs[:, :, 0, 0]
                V.memset(_re(gsd, [gsd.ap[1:][0], [S + 1, S]]), 0.0)
                V.tensor_mul(wc, Gs, wc)                  # sim (diag 0)
                V.tensor_scalar_mul(wd, iW, -0.1)
                actm_i = _re(actm, [actm.ap[1:][0], [1, S], [0, S]])
                actm_j = _re(actm, [actm.ap[1:][0], [0, S], [1, S]])
                V.tensor_mul(wd, wd, actm_i)
                V.tensor_mul(wd, wd, actm_j)
                V.tensor_mul(we, wd, wc)                  # E offdiag = -0.1*W*cond*sim
                wed = we[:, :, 0, 0]
                V.tensor_scalar(_re(wed, [wed.ap[1:][0], [S + 1, S]]),
                                _re(wed, [wed.ap[1:][0], [S + 1, S]]),
                                1.0, None, op0=OP.add)    # E diag = 1
                E9 = mid.tile([PB, T, S, 10], F32, tag="E9", name="E9")
                alpha_b = _re(alpha, [alpha.ap[1:][0], [1, S], [0, S]])
                V.tensor_mul(E9[:, :, :, 0:S], we, alpha_b)
                V.tensor_copy(E9[:, :, :, 8], beta)
                V.memset(E9[:, :, :, 9], 0.0)

                # ---- reconstruction ----
                m_dl = m10.rearrange("p t l d -> p t d l")
                mo_v = dview(do["m"], "sd")
                for i in range(S):
                    e_row = E9[:, :, i, :]
                    e_b = _re(e_row, [e_row.ap[1:][0], [0, D], [1, 10]])
                    P = big.tile([PB, T, D, 10], F32, tag="P", bufs=2, name="P")
                    omi = big.tile([PB, T, D], F32, tag="om", bufs=2, name="omi")
                    if i >= 5:
                        nc.gpsimd.tensor_mul(P, m_dl, e_b)
                    else:
                        V.tensor_mul(P, m_dl, e_b)
                    V.tensor_reduce(omi, P, axis=X, op=OP.add)
                    nc.sync.dma_start(out=mo_v[:, :, i, :], in_=omi)

                # ---- stores ----
                nc.sync.dma_start(out=dview(do["a"], "s"), in_=oa)
                nc.sync.dma_start(out=dview(do["g"], "s"), in_=og)
                nc.sync.dma_start(out=dview(do["mc"], "s"), in_=omc)
                nc.sync.dma_start(out=dview(do["ml"], "x"), in_=oml)
                nc.sync.dma_start(out=dview(do["ta"], "x"), in_=ota)
                nc.sync.dma_start(out=dview(do["ms"], "x"), in_=oms)
    return nc



def _split_waits(nc, max_waits=1):
    """This walrus build accepts at most one sync-wait per instruction; hoist
    excess waits onto dedicated Drain carriers inserted just before."""
    wid = [0]
    for blk in nc.m.functions[0].blocks:
        out = []
        for inst in blk.instructions:
            si = inst.sync_info
            w = list(si.on_wait or []) if si else []
            if len(w) > max_waits:
                rest, keep = w[:-max_waits], w[-max_waits:]
                for chunk_start in range(0, len(rest), max_waits):
                    chunk = rest[chunk_start:chunk_start + max_waits]
                    d = mybir.InstDrain(name=f"WSPLIT-{wid[0]}", ins=[], outs=[])
                    wid[0] += 1
                    d.engine = inst.engine
                    d.sync_info = mybir.SyncInfo(on_wait=chunk, on_update=[])
                    out.append(d)
                si.on_wait = keep
            out.append(inst)
        blk.instructions[:] = out


_NC_CACHE = {}


def _run(inputs, nblk, trace=False):
    dtv = float(np.asarray(inputs["dt"]))
    key = (nblk, dtv)
    if key not in _NC_CACHE:
        nc_new = build(nblk, dtv)
        _split_waits(nc_new)
        _NC_CACHE[key] = nc_new
    nc = _NC_CACHE[key]
    Bc = PB * T * nblk
    B = Bc * N_CORES
    names = ["memory_slots", "slot_activities", "slot_gates", "gate_thresholds",
             "refresh_strengths", "maintenance_currents", "gate_signals",
             "refresh_signals", "interference_matrix", "inputs"]
    in_maps = []
    for c in range(N_CORES):
        sl = slice(c * Bc, (c + 1) * Bc)
        in_maps.append({nm: np.ascontiguousarray(np.asarray(inputs[nm][sl], dtype=np.float32))
                        for nm in names})
    res = run_bass_kernel_spmd(nc, in_maps, core_ids=list(range(N_CORES)), trace=trace)
    outs = []
    for nm, shp in (("out_m", (B, S, D)), ("out_a", (B, S)), ("out_g", (B, S)),
                    ("out_mc", (B, S)), ("out_ml", (B,)), ("out_ta", (B,)), ("out_ms", (B,))):
        outs.append(np.concatenate([res.results[c][nm] for c in range(N_CORES)], axis=0).reshape(shp))
    return tuple(outs), res


def kernel(**inputs):
    (m, a, g, mc, ml, ta, ms), _ = _run(inputs, nblk=4)
    return m, a, g, mc, ml, ta, ms
